# revision 3
# baseline (speedup 1.0000x reference)
"""Grouped-scale dequant GEMM (AxCoreLinearFP16) on 8 Trainium2 NeuronCores — v2.

y[b,s,o] = sum_i x[b,s,i] * (weight[o,i] * scales[o, i//128])

Data-parallel over flattened (b*s) rows: each core gets [1024, 4096] of x and
the full weight/scales. Per core:
  - x^T resident in SBUF via two DMA transposes on the sync queue (all
    transposed DMAs share one queue: the XBAR transpose unit is stateful
    and concurrent transpose streams from two queues corrupt each other).
  - w^T o-panels (512 wide) DMA-transposed in two 2 MiB chunks, prefetch
    distance 2 (wraw bufs=3).
  - dequant: scales^T [32, 4096] built on-chip once (PE transposes); each
    (o-chunk, k-group) scales row is broadcast across 128 partitions by a
    one-hot row-selector matmul (stationary E[:, g, :], K=32, vs scT — no
    per-panel scales DMA, no DRAM bounce), then one DVE multiply applies it
    to the w^T chunk in place. Broadcast+mul quartets are interleaved
    between GEMM m-groups so the PE never stalls on the psb/DVE interlock.
  - PE matmul accumulates 32 k-chunks into PSUM [128, 512] fp32; ACT
    casting copy evicts; y DMA'd out on the scalar queue.

Environment workarounds (walrus here): only one sync-wait per instruction
(extra waits peeled onto same-engine NoOps); InstPartitionBroadcast and
--enable-ldw-opt do not codegen; both avoided.

Self-contained: hardcodes shapes from the problem spec.
"""

import sys

for _p in ("/opt/trn_rl_repo",):
    if _p not in sys.path:
        sys.path.insert(0, _p)

from contextlib import ExitStack

import numpy as np

import concourse.bass as bass
import concourse.mybir as mybir
import concourse.tile as tile
import bass_rust
from concourse.masks import make_identity


FP16 = mybir.dt.float16
FP32 = mybir.dt.float32

P = 128
NCORES = 8
B, S, IN, OUT = 4, 2048, 4096, 4096
GROUP = 128
M = B * S // NCORES          # 1024 rows of x per core
KO = IN // P                 # 32 k-chunks == quant groups
OC = 512                     # o-chunk (matmul free dim)
NOC = OUT // OC              # 8
MT = M // P                  # 8 m-tiles

_RUNNER = None


def _split_multiwait_insts(nc):
    """This env's walrus CoreV3 codegen accepts only one sync-wait per
    instruction; peel extra waits onto same-engine NoOps inserted before."""
    ctr = 0
    for f in nc.m.functions:
        for bb in f.blocks:
            new = []
            for inst in bb.instructions:
                si = inst.sync_info
                if si is not None and si.on_wait and len(si.on_wait) > 1:
                    waits = list(si.on_wait)
                    for w in waits[:-1]:
                        ctr += 1
                        new.append(bass_rust.InstNoOp(
                            name=f"I-waitsplit-{ctr}",
                            engine=inst.engine,
                            sync_info=bass_rust.SyncInfo(on_wait=[w], on_update=[]),
                        ))
                    inst.sync_info = bass_rust.SyncInfo(
                        on_wait=[waits[-1]], on_update=list(si.on_update or [])
                    )
                new.append(inst)
            bb.instructions = new
    return ctr


def _build(split_waits=True, reps=1, ilv=True, xsplit=True):
    nc = bass.Bass()
    x = nc.declare_dram_parameter("x", [M, IN], FP16, isOutput=False)
    w = nc.declare_dram_parameter("w", [OUT, IN], FP16, isOutput=False)
    s = nc.declare_dram_parameter("s", [OUT, KO], FP16, isOutput=False)
    y = nc.declare_dram_parameter("y", [M, OUT], FP16, isOutput=True)

    with tile.TileContext(nc) as tc, ExitStack() as ctx:
        const = ctx.enter_context(tc.tile_pool(name="const", bufs=1))
        scps = ctx.enter_context(tc.tile_pool(name="scps", bufs=2, space="PSUM"))
        xTp = ctx.enter_context(tc.tile_pool(name="xTp", bufs=1))
        wraw = ctx.enter_context(tc.tile_pool(name="wraw", bufs=3))
        psb_pool = ctx.enter_context(tc.tile_pool(name="psb", bufs=2, space="PSUM"))
        ystg = ctx.enter_context(tc.tile_pool(name="ystg", bufs=4))
        psum = ctx.enter_context(tc.tile_pool(name="psum", bufs=4, space="PSUM"))

        # scales^T on-chip: scT[g, o] = s[o, g], via 32 PE transposes.
        ident = const.tile([P, P], FP16)
        make_identity(nc, ident)
        snat = const.tile([P, OUT // P, KO], FP16)
        sv = s[:, :].rearrange("(oo p) g -> p oo g", p=P)
        nc.gpsimd.dma_start(out=snat[:], in_=sv)
        scT = const.tile([P, OUT], FP16)
        nc.vector.memset(scT[:], 0.0)
        for o2 in range(OUT // P):
            pst = scps.tile([KO, P], FP16, tag="pst")
            nc.tensor.transpose(pst[:], snat[:, o2, :], ident[:])
            nc.scalar.copy(out=scT[0:KO, o2 * P:(o2 + 1) * P], in_=pst[:])

        # One-hot row selectors: E[gp, g, c] = 1 iff gp == g (any c).
        # matmul(psb, E[:, g, :], scT[:, osl]) => psb[p, c] = scT[g, osl+c].
        E = const.tile([P, KO, P], FP16)
        nc.gpsimd.memset(E[:], 0.0)
        nc.gpsimd.affine_select(
            out=E[:], in_=E[:],
            compare_op=mybir.AluOpType.not_equal,
            fill=1.0, base=0,
            # expr = p - g; == 0 on the (p == g) planes -> fill 1.0 there
            pattern=[[-1, KO], [0, P]],
            channel_multiplier=1,
        )

        # x^T resident: xT[p, ko, m] = x[m, ko*128+p].  With xsplit, x is
        # loaded as two half-M tiles so the first GEMMs wait on only half
        # the transpose traffic, and panel-0 w loads are sandwiched between.
        MH = M // 2
        if xsplit:
            xTa = xTp.tile([P, KO, MH], FP16, tag="xa")
            xTb = xTp.tile([P, KO, MH], FP16, tag="xb")

            def xTsel(ko, m):
                t = xTa if m < MT // 2 else xTb
                mm = m % (MT // 2)
                return t[:, ko, mm * P:(mm + 1) * P]
        else:
            xT = xTp.tile([P, KO, M], FP16)

            def xTsel(ko, m):
                return xT[:, ko, m * P:(m + 1) * P]

        KH = KO // 2

        def emit_wdma(oc):
            osl = slice(oc * OC, (oc + 1) * OC)
            wr3 = wraw.tile([P, KO, OC], FP16, tag="wraw", name="wr3")
            for g in range(2):
                kg = slice(g * KH, (g + 1) * KH)
                nc.sync.dma_start_transpose(
                    out=wr3[:, kg, :], in_=w[osl, g * KH * P:(g + 1) * KH * P])
            return wr3

        def emit_deq(oc, wr3, kos):
            osl = slice(oc * OC, (oc + 1) * OC)
            for ko in kos:
                psb = psb_pool.tile([P, OC], FP32, tag="psb", name="psb")
                nc.tensor.matmul(psb[:], E[:, ko, :], scT[:, osl],
                                 start=True, stop=True)
                nc.vector.tensor_mul(wr3[:, ko, :], wr3[:, ko, :], psb[:])

        def emit_compute(oc, wr3, interleave):
            osl = slice(oc * OC, (oc + 1) * OC)
            for m in range(MT):
                pt = psum.tile([P, OC], FP32, name="pt")
                for ko in range(KO):
                    nc.tensor.matmul(
                        pt[:],
                        xTsel(ko, m),
                        wr3[:, ko, :],
                        start=(ko == 0),
                        stop=(ko == KO - 1),
                    )
                yt = ystg.tile([P, OC], FP16, name="yt")
                nc.scalar.copy(out=yt[:], in_=pt[:])
                nc.scalar.dma_start(out=y[m * P:(m + 1) * P, osl], in_=yt[:])
                if interleave is not None:
                    noc2, nwr3 = interleave
                    emit_deq(noc2, nwr3, range(4 * m, 4 * m + 4))

        for rr in range(reps):
            if xsplit:
                nc.sync.dma_start_transpose(out=xTa[:], in_=x[0:MH, :])
                wr_a = emit_wdma(0)
                emit_deq(0, wr_a, range(KO))
                wr_b = emit_wdma(1)
                emit_deq(1, wr_b, range(KO))
                nc.sync.dma_start_transpose(out=xTb[:], in_=x[MH:M, :])
            else:
                xc = KO // 2
                for i in range(2):
                    nc.sync.dma_start_transpose(
                        out=xT[:, i * xc:(i + 1) * xc, :],
                        in_=x[:, i * xc * P:(i + 1) * xc * P])
                wr_a = emit_wdma(0)
                emit_deq(0, wr_a, range(KO))
                wr_b = emit_wdma(1)
                emit_deq(1, wr_b, range(KO))

            panels = {0: wr_a, 1: wr_b}
            for oc in range(NOC):
                nxt = oc + 2
                if nxt < NOC:
                    panels[nxt] = emit_wdma(nxt)
                    if ilv:
                        emit_compute(oc, panels[oc], (nxt, panels[nxt]))
                    else:
                        emit_deq(nxt, panels[nxt], range(KO))
                        emit_compute(oc, panels[oc], None)
                else:
                    emit_compute(oc, panels[oc], None)
                del panels[oc]

    if split_waits:
        _split_multiwait_insts(nc)
    return nc


def _get_runner():
    """Compile once; return a reusable callable mapping per-core input maps
    to per-core output maps."""
    global _RUNNER
    if _RUNNER is not None:
        return _RUNNER

    import jax
    from jax.experimental.shard_map import shard_map
    from jax.sharding import Mesh, PartitionSpec
    from concourse import bass2jax

    nc = _build()
    bass2jax.install_neuronx_cc_hook()

    partition_name = nc.partition_id_tensor.name if nc.partition_id_tensor else None
    in_names, out_names, out_avals, zero_shapes = [], [], [], []
    for alloc in nc.m.functions[0].allocations:
        if not isinstance(alloc, mybir.MemoryLocationSet):
            continue
        name = alloc.memorylocations[0].name
        if alloc.kind == "ExternalInput":
            if name != partition_name:
                in_names.append(name)
        elif alloc.kind == "ExternalOutput":
            shape = tuple(alloc.tensor_shape)
            dtype = mybir.dt.np(alloc.dtype)
            out_names.append(name)
            out_avals.append(jax.core.ShapedArray(shape, dtype))
            zero_shapes.append((shape, dtype))
    n_params = len(in_names)
    n_outs = len(out_names)
    all_names = in_names + out_names
    if partition_name is not None:
        all_names = all_names + [partition_name]
    donate = tuple(range(n_params, n_params + n_outs))

    def _make_body(reps):
        def _body(*args):
            ins = list(args[:n_params])
            outs = list(args[n_params:n_params + n_outs])
            for _ in range(reps):
                operands = ins + outs
                if partition_name is not None:
                    operands.append(bass2jax.partition_id_tensor())
                outs = list(bass2jax._bass_exec_p.bind(
                    *operands,
                    out_avals=tuple(out_avals),
                    in_names=tuple(all_names),
                    out_names=tuple(out_names),
                    lowering_input_output_aliases=(),
                    sim_require_finite=True,
                    sim_require_nnan=True,
                    nc=nc,
                ))
            return tuple(outs)
        return _body

    devices = jax.devices()[:NCORES]
    mesh = Mesh(np.asarray(devices), ("core",))

    def _make_exec(reps):
        return jax.jit(
            shard_map(
                _make_body(reps),
                mesh=mesh,
                in_specs=(PartitionSpec("core"),) * (n_params + n_outs),
                out_specs=(PartitionSpec("core"),) * n_outs,
                check_rep=False,
            ),
            donate_argnums=donate,
            keep_unused=True,
        )

    sharded = _make_exec(1)
    _exec_cache = {1: sharded}
    from jax.sharding import NamedSharding
    shard = NamedSharding(mesh, PartitionSpec("core"))

    class Runner:
        def __init__(self):
            self.in_names = in_names
            self.out_names = out_names

        def put_inputs(self, in_maps):
            import jax as _jax
            concat_in = [
                np.concatenate([np.asarray(m[name]) for m in in_maps], axis=0)
                for name in in_names
            ]
            return [_jax.device_put(a, shard) for a in concat_in]

        def fresh_outs(self):
            import jax as _jax
            return [
                _jax.device_put(np.zeros((NCORES * sh[0], *sh[1:]), dt), shard)
                for sh, dt in zero_shapes
            ]

        def exec_dev(self, dev_in, dev_outs, reps=1):
            if reps not in _exec_cache:
                _exec_cache[reps] = _make_exec(reps)
            return _exec_cache[reps](*dev_in, *dev_outs)

        def run(self, in_maps):
            dev_in = self.put_inputs(in_maps)
            out_arrs = self.exec_dev(dev_in, self.fresh_outs())
            return [
                {
                    name: np.asarray(out_arrs[i]).reshape(
                        NCORES, *out_avals[i].shape)[c]
                    for i, name in enumerate(out_names)
                }
                for c in range(NCORES)
            ]

    _RUNNER = Runner()
    return _RUNNER


def kernel(x, weight, scales):
    runner = _get_runner()
    xf = np.ascontiguousarray(np.asarray(x, dtype=np.float16).reshape(B * S, IN))
    w = np.ascontiguousarray(np.asarray(weight, dtype=np.float16))
    s = np.ascontiguousarray(np.asarray(scales, dtype=np.float16))
    in_maps = [
        {"x": xf[c * M:(c + 1) * M], "w": w, "s": s} for c in range(NCORES)
    ]
    outs = runner.run(in_maps)
    yf = np.concatenate([outs[c]["y"] for c in range(NCORES)], axis=0)
    return yf.reshape(B, S, OUT).astype(np.float16)
